# revision 85
# baseline (speedup 1.0000x reference)
"""Llama GQA attention (B=1, S=2048, E=4096, H=32, KV=8, D=128) on 8 trn2 cores.

Sharding: tensor-parallel over KV groups. Core c owns kv head c and q heads
4c..4c+3: wq/wk/wv output-dim shards, wo input-dim shard. Each core computes a
partial [S, E] output (bf16); host sums the 8 partials and adds bo.

Data plane is bf16 (PE runs bf16 at 1 cycle/row for any moving width; DMA
traffic halves vs f32). PSUM accumulation stays f32; RoPE uses f32 cos/sin.
1/sqrt(D) is folded into wq on the host, so q and k share one cos/sin pair.

Per core, everything transposed [feature, seq]:
  phase 1 (per 512-seq chunk, two passes over resident x tiles so the acc
  PSUM banks double-buffer): q = wq_c.T @ x.T -> 4x [128, S]; k, v -> [128, S].
  RoPE via partition-swapped multiply (host passes sign-adjusted sin).
  v transposed via PE into vTo [k, kt, 129] with a constant 1.0 in col 128.
  phase 2: scoresT tile [k 128, q<=512] = kr.T-matmul; diagonal tiles compute
  only the visible q range and add one shared [128,128] triangle mask; Exp on
  ACT -> P bf16. AV is flipped: stationary = P[:, qs*128:+128], moving =
  vTo[:, kt, 0:129] -> av2 [q 128, 129] accumulates over kt in PSUM, col 128
  = softmax denominator for free. Per-partition reciprocal + scalar-mul
  normalize, PE-transpose back to [D, q] for the o-projection.
  phase 3: out[q, E] += o_tile.T @ wo, interleaved one chunk behind attention
  so the PE keeps busy while ACT catches up on Exp.
PSUM is managed as 8 explicit bank tags in one pool (no pool-boundary stalls):
b0-b2 proj passA / scores+..., b3-b5 proj passB / AV accumulators,
b6-b7 v-transposes / o-proj.
"""

import sys

sys.path.insert(0, "/opt/trn_rl_repo")

import numpy as np
import ml_dtypes

import concourse.bass as bass  # noqa: F401
import concourse.bacc as bacc
import concourse.mybir as mybir
import concourse.tile as tile
from concourse.bass_utils import run_bass_kernel_spmd
from concourse.masks import make_identity

F32 = mybir.dt.float32
BF16 = mybir.dt.bfloat16
ADD = mybir.AluOpType.add
MULT = mybir.AluOpType.mult
EXP = mybir.ActivationFunctionType.Exp
BF = ml_dtypes.bfloat16

B, S, E = 1, 2048, 4096
H, KV, D = 32, 8, 128
NCORES = 8
HPC = H // NCORES          # 4 q heads per core
ET = E // 128              # 32 contraction tiles
SC = S // 512              # 4 seq chunks of 512
KT = S // 128              # 16 k tiles of 128
ECH = E // 512             # 8 output E chunks
NEG = -1e9

SKIP = "S"
NOMASK = "N"

_build_cache = {}


def _build(classes, n_mtiles, use_bias):
    nc = bacc.Bacc(None, target_bir_lowering=False)

    xT = nc.declare_dram_parameter("xT", [E, S], BF16, isOutput=False)
    # wab: [q0|q1|v | q2|q3|k] so pass A's half loads first
    wab = nc.declare_dram_parameter("wab", [E, 6 * D], BF16, isOutput=False)
    wo = nc.declare_dram_parameter("wo", [HPC * D, E], BF16, isOutput=False)
    cos = nc.declare_dram_parameter("cos", [D, S], F32, isOutput=False)
    sinS = nc.declare_dram_parameter("sinS", [D, S], F32, isOutput=False)
    # causal triangle as a rank-128 product: triA.T @ triB = -1e9*(k-q)*[k>q]
    triA = nc.declare_dram_parameter("triA", [D, D], BF16, isOutput=False)
    triB = nc.declare_dram_parameter("triB", [D, D], BF16, isOutput=False)
    if n_mtiles:
        mtiles = nc.declare_dram_parameter(
            "mtiles", [n_mtiles * 128, 512], F32, isOutput=False)
    if use_bias:
        bq = nc.declare_dram_parameter("bq", [HPC * D], F32, isOutput=False)
        bk = nc.declare_dram_parameter("bk", [D], F32, isOutput=False)
        bv = nc.declare_dram_parameter("bv", [D], F32, isOutput=False)
    out = nc.declare_dram_parameter("out", [S, E], BF16, isOutput=True)

    wab_r = wab.rearrange("(t p) n -> p t n", p=128)
    wo_r = wo.rearrange("(t p) n -> p t n", p=128)
    xT_r = xT.rearrange("(t p) s -> p t s", p=128)

    with tile.TileContext(nc) as tc:
        with (
            tc.tile_pool(name="const", bufs=1) as cpool,
            tc.tile_pool(name="qkv", bufs=1) as qkvpool,
            tc.tile_pool(name="wts", bufs=1) as wpool,
            tc.tile_pool(name="xres", bufs=2) as xpool,
            tc.tile_pool(name="cs", bufs=2) as cspool,
            tc.tile_pool(name="tp", bufs=1) as tpool,
            tc.tile_pool(name="ps", bufs=6) as spool,
            tc.tile_pool(name="osb", bufs=2) as opool,
            tc.tile_pool(name="onrm", bufs=4) as onpool,
            tc.tile_pool(name="ob", bufs=4) as obpool,
            tc.tile_pool(name="psum", bufs=1, space="PSUM") as P,
        ):
            ident = cpool.tile([128, 128], BF16)
            make_identity(nc, ident)
            triA_sb = cpool.tile([128, 128], BF16)
            triB_sb = cpool.tile([128, 128], BF16)  # DMA'd after chunk-0 loads
            mt_sb = None
            if n_mtiles:
                mt_sb = cpool.tile([128, n_mtiles, 512], F32)
                nc.sync.dma_start(
                    out=mt_sb,
                    in_=mtiles.rearrange("(t p) n -> p t n", p=128))
            if use_bias:
                bq_sb = cpool.tile([128, HPC], F32)
                nc.sync.dma_start(out=bq_sb, in_=bq.rearrange("(h d) -> d h", d=128))
                bk_sb = cpool.tile([128, 1], F32)
                nc.sync.dma_start(out=bk_sb, in_=bk.rearrange("d -> d 1"))
                bv_sb = cpool.tile([128, 1], F32)
                nc.sync.dma_start(out=bv_sb, in_=bv.rearrange("d -> d 1"))

            # persistent activations
            qr = [qkvpool.tile([128, S], BF16, name=f"qr{h}", tag=f"qr{h}")
                  for h in range(HPC)]
            kr = qkvpool.tile([128, S], BF16, name="kr", tag="kr")
            vTo = qkvpool.tile([128, KT, 129], BF16, tag="vTo")  # [k%128, kt, D|1]
            nc.vector.memset(vTo[:, :, 128:129], 1.0)

            wab_sb = wpool.tile([128, ET, 6 * D], BF16)
            wo_sb = wpool.tile([128, HPC, E], BF16)

            # ---------------- phase 1: projections + RoPE + vT ----------------
            def rope(dst, acc, ct, st_, bias):
                src = acc
                if use_bias:
                    bsrc = tpool.tile([128, 512], F32, name="bsrc", tag="bsrc")
                    nc.vector.tensor_scalar_add(bsrc, acc, bias)
                    src = bsrc
                tmp = tpool.tile([128, 512], F32, name="tmp", tag="tmp")
                nc.vector.tensor_tensor(
                    out=tmp[0:64, :], in0=src[64:128, :], in1=st_[0:64, :], op=MULT)
                nc.vector.tensor_tensor(
                    out=tmp[64:128, :], in0=src[0:64, :], in1=st_[64:128, :], op=MULT)
                tmp2 = tpool.tile([128, 512], F32, name="tmp2", tag="tmp2")
                nc.vector.tensor_tensor(out=tmp2, in0=src, in1=ct, op=MULT)
                nc.vector.tensor_tensor(out=dst, in0=tmp2, in1=tmp, op=ADD)

            BA = ["b0", "b1", "b2"]
            BB = ["b3", "b4", "b5"]
            def emit_vtrans(c, vtmp):
                for j in range(4):
                    tb = "b6" if j % 2 == 0 else "b7"
                    tpsum = P.tile([128, 128], BF16, name="tp", tag=tb)
                    nc.tensor.transpose(
                        tpsum, vtmp[:, j * 128:(j + 1) * 128], ident)
                    nc.vector.tensor_copy(
                        out=vTo[:, c * 4 + j, 0:128], in_=tpsum)

            pend_vtrans = None  # chunk 0's v-transposes run in chunk 1
            for c in range(SC):
                ssl = slice(c * 512, c * 512 + 512)
                xc = xpool.tile([128, ET, 512], BF16, name="xc", tag="xc")
                # DMA in consumption order (HWDGE issue is ~625ns per
                # dma_start — few big DMAs, not many small ones)
                cq = cspool.tile([128, 512], F32, name="cq", tag="cq")
                sq = cspool.tile([128, 512], F32, name="sq", tag="sq")
                if c == 0:
                    # both weight halves per group: chunk 0 runs passes A+B
                    # interleaved so PE outpaces the DMA ramp
                    edges = [0, 1, 4, 8, 12, 16, 20, 24, 28, 32]
                    for gi in range(len(edges) - 1):
                        eg = slice(edges[gi], edges[gi + 1])
                        nc.sync.dma_start(
                            out=wab_sb[:, eg, :], in_=wab_r[:, eg, :])
                        nc.sync.dma_start(out=xc[:, eg, :], in_=xT_r[:, eg, ssl])
                        if gi == 4:
                            nc.sync.dma_start(out=cq, in_=cos[:, ssl])
                            nc.sync.dma_start(out=sq, in_=sinS[:, ssl])
                    nc.sync.dma_start(out=triA_sb, in_=triA[:, :])
                    nc.sync.dma_start(out=triB_sb, in_=triB[:, :])
                else:
                    for g in range(4):
                        eg = slice(g * 8, g * 8 + 8)
                        nc.sync.dma_start(out=xc[:, eg, :], in_=xT_r[:, eg, ssl])
                    nc.sync.dma_start(out=cq, in_=cos[:, ssl])
                    nc.sync.dma_start(out=sq, in_=sinS[:, ssl])
                    if c in (1, 2):
                        for hg in ((0, 1) if c == 1 else (2, 3)):
                            nc.sync.dma_start(
                                out=wo_sb[:, hg:hg + 1, :],
                                in_=wo_r[:, hg:hg + 1, :])

                accs = [P.tile([128, 512], F32, name=f"pa{i}", tag=BA[i])
                        for i in range(3)]
                accs2 = [P.tile([128, 512], F32, name=f"pb{i}", tag=BB[i])
                         for i in range(3)]
                def make_vtmp(acc):
                    vt = tpool.tile([128, 512], BF16, name="vtmp", tag="vtmp",
                                    bufs=2)
                    if use_bias:
                        nc.vector.tensor_scalar_add(vt, acc, bv_sb[:, 0:1])
                    else:
                        nc.scalar.copy(out=vt, in_=acc)
                    return vt

                if c == 0:
                    # single fused pass, e-major: chunk 0 is DMA-paced so the
                    # matmuls must chase the per-e loads
                    for e in range(ET):
                        st, sp = (e == 0), (e == ET - 1)
                        for ai, col in enumerate((0, 128, 256)):
                            nc.tensor.matmul(
                                accs[ai], wab_sb[:, e, col:col + 128],
                                xc[:, e, :], start=st, stop=sp)
                        for ai, col in enumerate((384, 512, 640)):
                            nc.tensor.matmul(
                                accs2[ai], wab_sb[:, e, col:col + 128],
                                xc[:, e, :], start=st, stop=sp)
                    rope(qr[0][:, ssl], accs[0], cq, sq,
                         bq_sb[:, 0:1] if use_bias else None)
                    rope(qr[1][:, ssl], accs[1], cq, sq,
                         bq_sb[:, 1:2] if use_bias else None)
                    vtmp = make_vtmp(accs[2])
                    pend_vtrans = (0, vtmp)
                    rope(kr[:, ssl], accs2[2], cq, sq,
                         bk_sb[:, 0:1] if use_bias else None)
                    rope(qr[2][:, ssl], accs2[0], cq, sq,
                         bq_sb[:, 2:3] if use_bias else None)
                    rope(qr[3][:, ssl], accs2[1], cq, sq,
                         bq_sb[:, 3:4] if use_bias else None)
                else:
                    # acc-major: each accumulator finishes early so its RoPE /
                    # copy overlaps the next accumulation instead of tailing
                    def acc_loop(acc, col):
                        for e in range(ET):
                            nc.tensor.matmul(
                                acc, wab_sb[:, e, col:col + 128], xc[:, e, :],
                                start=(e == 0), stop=(e == ET - 1))
                    acc_loop(accs[2], 256)                     # v
                    vtmp = make_vtmp(accs[2])
                    if pend_vtrans is not None:
                        emit_vtrans(*pend_vtrans)
                        pend_vtrans = None
                    acc_loop(accs[0], 0)                       # q0
                    rope(qr[0][:, ssl], accs[0], cq, sq,
                         bq_sb[:, 0:1] if use_bias else None)
                    acc_loop(accs[1], 128)                     # q1
                    rope(qr[1][:, ssl], accs[1], cq, sq,
                         bq_sb[:, 1:2] if use_bias else None)
                    emit_vtrans(c, vtmp)
                    if c == SC - 1:
                        acc_loop(accs2[2], 640)                # k first
                        rope(kr[:, ssl], accs2[2], cq, sq,
                             bk_sb[:, 0:1] if use_bias else None)
                        acc_loop(accs2[0], 384)                # q2
                        acc_loop(accs2[1], 512)                # q3
                        # defer the last two RoPEs: they would block q-chunk-0
                        # attention's DVE work (in-order engine)
                        pend_ropes = [
                            (qr[2][:, ssl], accs2[0],
                             bq_sb[:, 2:3] if use_bias else None),
                            (qr[3][:, ssl], accs2[1],
                             bq_sb[:, 3:4] if use_bias else None),
                        ]
                        pend_rope_cs = (cq, sq)
                    else:
                        acc_loop(accs2[0], 384)                # q2
                        rope(qr[2][:, ssl], accs2[0], cq, sq,
                             bq_sb[:, 2:3] if use_bias else None)
                        acc_loop(accs2[1], 512)                # q3
                        rope(qr[3][:, ssl], accs2[1], cq, sq,
                             bq_sb[:, 3:4] if use_bias else None)
                        acc_loop(accs2[2], 640)                # k
                        rope(kr[:, ssl], accs2[2], cq, sq,
                             bk_sb[:, 0:1] if use_bias else None)

            # ------------- phases 2+3: attention (+interleaved o-proj) -------
            o_bufs = [None, None]  # [qc%2] -> list of 4 o_sb tiles

            def oproj_pair(qcp, qs, ec0, slot=("b6", "b7"),
                           acts=(False, False), split_dma=False):
                """Two consecutive ec units sharing one output DMA."""
                osrc = o_bufs[qcp % 2]
                q0 = qcp * 512 + qs * 128
                ob = obpool.tile([128, 1024], BF16, name="ob", tag="ob")
                for k in range(2):
                    ec = ec0 + k
                    op = P.tile([128, 512], F32, name="op", tag=slot[k])
                    for hh in range(HPC):
                        nc.tensor.matmul(
                            op,
                            osrc[hh][:, qs * 128:(qs + 1) * 128],
                            wo_sb[:, hh, ec * 512:(ec + 1) * 512],
                            start=(hh == 0), stop=(hh == HPC - 1),
                            skip_group_check=True)
                    if acts[k]:
                        nc.scalar.copy(out=ob[:, k * 512:(k + 1) * 512], in_=op)
                    else:
                        nc.vector.tensor_copy(
                            out=ob[:, k * 512:(k + 1) * 512], in_=op)
                    if split_dma:
                        nc.sync.dma_start(
                            out=out[q0:q0 + 128, ec * 512:(ec + 1) * 512],
                            in_=ob[:, k * 512:(k + 1) * 512])
                if not split_dma:
                    nc.sync.dma_start(
                        out=out[q0:q0 + 128, ec0 * 512:(ec0 + 2) * 512], in_=ob)

            for qc in range(SC):
                # qc0 runs before the deferred chunk-3 RoPE tail frees
                # b3/b4/b5: keep it entirely off those banks (qs3 reuses b2
                # after qs0's tail releases it)
                AVB = (["b2", "b6", "b7", "b2"] if qc == 0
                       else ["b2", "b3", "b4", "b5"])
                cls = classes[qc]
                vis = [kt for kt in range(KT) if cls[kt] != SKIP]
                o_cur = [opool.tile([128, 512], BF16, name=f"o{h}", tag=f"o{h}")
                         for h in range(HPC)]
                o_bufs[qc % 2] = o_cur
                for h in range(HPC):
                    pend_oproj = list(range(ECH)) if qc > 0 else []
                    ob_half = [None]  # open ob tile for the current pair

                    def emit_op_unit(ec):
                        """One o-proj ec unit; pairs share an ob tile+DMA."""
                        osrc = o_bufs[(qc - 1) % 2]
                        q0 = (qc - 1) * 512 + h * 128
                        if ec % 2 == 0:
                            ob_half[0] = obpool.tile(
                                [128, 1024], BF16, name="ob", tag="ob")
                        ob = ob_half[0]
                        op = P.tile([128, 512], F32, name="op",
                                    tag="b6" if ec % 2 == 0 else "b7")
                        for hh in range(HPC):
                            nc.tensor.matmul(
                                op,
                                osrc[hh][:, h * 128:(h + 1) * 128],
                                wo_sb[:, hh, ec * 512:(ec + 1) * 512],
                                start=(hh == 0), stop=(hh == HPC - 1),
                                skip_group_check=True)
                        k = ec % 2
                        if k == 0:
                            nc.scalar.copy(
                                out=ob[:, k * 512:(k + 1) * 512], in_=op)
                        else:
                            nc.vector.tensor_copy(
                                out=ob[:, k * 512:(k + 1) * 512], in_=op)
                        if k == 1:
                            nc.sync.dma_start(
                                out=out[q0:q0 + 128,
                                        (ec - 1) * 512:(ec + 1) * 512],
                                in_=ob)
                    # per-qs AV accumulation state: kt lists
                    avkts = [[kt for kt in vis
                              if not (isinstance(cls[kt], tuple)
                                      and cls[kt][0] == "T"
                                      and cls[kt][1] > qs)]
                             for qs in range(4)]
                    avseen = [0, 0, 0, 0]
                    av2 = [P.tile([128, 129], F32, name=f"av{qs}", tag=AVB[qs])
                           for qs in range(4)]
                    def emit_qs_tail(qs):
                        rl = onpool.tile([128, 1], F32, name="rl", tag="rl")
                        nc.vector.reciprocal(rl, av2[qs][:, 128:129])
                        otn = onpool.tile([128, 128], BF16, name="otn",
                                          tag="otn")
                        nc.vector.tensor_scalar_mul(otn, av2[qs][:, 0:128], rl)
                        tp2 = P.tile([128, 128], BF16, name="tp2", tag=AVB[qs])
                        nc.tensor.transpose(tp2, otn, ident)
                        if qc >= 2:  # ACT is exp-saturated in late chunks
                            nc.vector.tensor_copy(
                                out=o_cur[h][:, qs * 128:(qs + 1) * 128],
                                in_=tp2)
                        else:
                            nc.scalar.copy(
                                out=o_cur[h][:, qs * 128:(qs + 1) * 128],
                                in_=tp2)

                    def emit_av(kt, p):
                        for qs in range(4):
                            if kt not in avkts[qs]:
                                continue
                            first = avseen[qs] == 0
                            avseen[qs] += 1
                            last = avseen[qs] == len(avkts[qs])
                            nc.tensor.matmul(
                                av2[qs],
                                p[:, qs * 128:(qs + 1) * 128],
                                vTo[:, kt, :],
                                start=first, stop=last,
                                skip_group_check=True)
                            if last:
                                if qc == 0:
                                    emit_qs_tail(qs)  # frees b2 for qs3
                                else:
                                    tails.append(qs)

                    pops = {}
                    for k in range(ECH):
                        pops.setdefault((k * len(vis)) // ECH, 0)
                        pops[(k * len(vis)) // ECH] += 1
                    prev = None  # (kt, p) — AV runs one tile behind exp
                    tails = []  # qs normalize/transpose, deferred one unit
                    for i, kt in enumerate(vis):
                        if tails:
                            emit_qs_tail(tails.pop(0))
                        cl = cls[kt]
                        q0 = cl[1] * 128 if (isinstance(cl, tuple)
                                             and cl[0] == "T") else 0
                        stp = P.tile([128, 512], F32, name="st",
                                     tag="b0" if i % 2 == 0 else "b1")
                        nc.tensor.matmul(
                            stp[:, q0:512],
                            kr[:, kt * 128:(kt + 1) * 128],
                            qr[h][:, qc * 512 + q0: qc * 512 + 512],
                            start=True, stop=True, skip_group_check=True)
                        if isinstance(cl, tuple) and cl[0] == "T":
                            nc.tensor.matmul(
                                stp[:, q0:q0 + 128], triA_sb, triB_sb,
                                start=False, stop=True, skip_group_check=True)
                        elif isinstance(cl, tuple) and cl[0] == "M":
                            nc.vector.tensor_tensor(
                                out=stp, in0=stp, in1=mt_sb[:, cl[1], :],
                                op=ADD)
                        p = spool.tile([128, 512], BF16, name="p", tag="p")
                        nc.scalar.activation(
                            out=p[:, q0:512], in_=stp[:, q0:512], func=EXP)
                        if prev is not None:
                            emit_av(*prev)
                        prev = (kt, p)
                        for _ in range(pops.get(i, 0)):
                            if pend_oproj:
                                emit_op_unit(pend_oproj.pop(0))
                    emit_av(*prev)
                    while tails or pend_oproj:
                        if pend_oproj:
                            emit_op_unit(pend_oproj.pop(0))
                        if tails:
                            emit_qs_tail(tails.pop(0))
                if qc == 0:
                    cqd, sqd = pend_rope_cs
                    for dst, acc, bias in pend_ropes:
                        rope(dst, acc, cqd, sqd, bias)
            # final o-proj for the last chunk: six banks, copies alternate
            # DVE/ACT (nothing else runs here)
            FB = ["b0", "b1", "b2", "b3", "b4", "b5"]
            for qs in range(4):
                for pi, ec0 in enumerate(range(0, ECH, 2)):
                    u = qs * 4 + pi
                    oproj_pair(SC - 1, qs, ec0,
                               slot=(FB[(2 * u) % 6], FB[(2 * u + 1) % 6]),
                               acts=(False, True), split_dma=(u >= 14))

    nc.finalize()
    return nc


def _host_prep(x, mask, position_ids, wq, bq, wk, bk, wv, bv, wo, bo):
    scale = 1.0 / np.sqrt(np.float32(D))
    xT = np.ascontiguousarray(x.reshape(S, E).T).astype(BF)
    wq_s = (wq * scale).astype(np.float32)
    wk_b = wk.astype(np.float32)
    wv_b = wv.astype(np.float32)
    wo_b = wo.astype(BF)

    pos = position_ids.reshape(S).astype(np.float32)
    inv_freq = 1.0 / (10000.0 ** (np.arange(0, D, 2, dtype=np.float32) / D))
    freqs = np.outer(pos, inv_freq)                     # [S, D/2]
    emb = np.concatenate([freqs, freqs], axis=1)        # [S, D]
    cosT = np.ascontiguousarray(np.cos(emb).astype(np.float32).T)
    sin = np.sin(emb).astype(np.float32)
    sin[:, : D // 2] *= -1.0                            # sign for partition swap
    sinT = np.ascontiguousarray(sin.T)

    maskT = np.ascontiguousarray(mask.reshape(S, S).T)
    # canonical 128x128 triangle: T[k, q] = 0 if q >= k else NEG
    ktri = np.arange(128)[:, None]
    qtri = np.arange(128)[None, :]
    tri = np.where(qtri >= ktri, 0.0, NEG).astype(np.float32)
    # rank-128 factors: (triA.T @ triB)[k, q] = -s^2 (k - q) for k > q, 0 else
    sfac = np.float32(np.sqrt(1e9))
    mtri = np.arange(128)
    triA = np.where(mtri[:, None] < mtri[None, :], -sfac, 0.0).astype(BF)
    triB = np.where(mtri[:, None] >= mtri[None, :], sfac, 0.0).astype(BF)

    classes = []
    muniq = []      # unique general mask tiles
    mkeys = {}

    def mref(t):
        key = t.tobytes()
        if key not in mkeys:
            mkeys[key] = len(muniq)
            muniq.append(t)
        return ("M", mkeys[key])

    for qc in range(SC):
        row = []
        for kt in range(KT):
            t = maskT[kt * 128:(kt + 1) * 128, qc * 512:qc * 512 + 512]
            if np.all(t <= -1e8):
                row.append(SKIP)
                continue
            if np.all(t == 0.0):
                row.append(NOMASK)
                continue
            j = kt - 4 * qc
            if 0 <= j <= 3:
                q0 = j * 128
                ok = (q0 == 0 or np.all(t[:, :q0] <= -1e8))
                ok = ok and np.array_equal(t[:, q0:q0 + 128], tri)
                ok = ok and (q0 + 128 == 512 or np.all(t[:, q0 + 128:] == 0.0))
                if ok:
                    row.append(("T", j))
                    continue
            row.append(mref(t))
        if all(c == SKIP for c in row):       # fully-masked rows: keep finite
            row = [mref(maskT[kt * 128:(kt + 1) * 128,
                              qc * 512:qc * 512 + 512]) for kt in range(KT)]
        classes.append(tuple(row))
    classes = tuple(classes)

    mtiles = np.concatenate(muniq, axis=0) if muniq else None
    use_bias = bool(np.any(bq) or np.any(bk) or np.any(bv))
    return (xT, wq_s, wk_b, wv_b, wo_b, cosT, sinT, triA, triB, mtiles,
            classes, use_bias)


def kernel(x, mask, position_ids, wq, bq, wk, bk, wv, bv, wo, bo):
    (xT, wq_s, wk_b, wv_b, wo_b, cosT, sinT, triA, triB, mtiles,
     classes, use_bias) = _host_prep(
        x, mask, position_ids, wq, bq, wk, bk, wv, bv, wo, bo)

    n_mtiles = 0 if mtiles is None else mtiles.shape[0] // 128
    key = (classes, n_mtiles, use_bias)
    if key not in _build_cache:
        _build_cache[key] = _build(classes, n_mtiles, use_bias)
    nc = _build_cache[key]

    in_maps = []
    for c in range(NCORES):
        qsl = slice(c * HPC * D, (c + 1) * HPC * D)
        ksl = slice(c * D, (c + 1) * D)
        wqc = wq_s[:, qsl]
        # [q0|q1|v | q2|q3|k]: pass-A half first
        wab = np.ascontiguousarray(np.concatenate(
            [wqc[:, 0:256], wv_b[:, ksl], wqc[:, 256:512], wk_b[:, ksl]],
            axis=1)).astype(BF)
        m = {
            "xT": xT,
            "wab": wab,
            "wo": np.ascontiguousarray(wo_b[qsl, :]),
            "cos": cosT, "sinS": sinT, "triA": triA, "triB": triB,
        }
        if mtiles is not None:
            m["mtiles"] = mtiles
        if use_bias:
            m["bq"] = np.ascontiguousarray(bq[qsl]).astype(np.float32)
            m["bk"] = np.ascontiguousarray(bk[ksl]).astype(np.float32)
            m["bv"] = np.ascontiguousarray(bv[ksl]).astype(np.float32)
        in_maps.append(m)

    res = run_bass_kernel_spmd(nc, in_maps, list(range(NCORES)))
    kernel._last_results = res

    acc = res.results[0]["out"].astype(np.float32)
    for c in range(1, NCORES):
        acc = acc + res.results[c]["out"].astype(np.float32)
    acc = acc + bo[None, :]
    return acc.reshape(B, S, E).astype(np.float32)


# revision 87
# speedup vs baseline: 1.0007x; 1.0007x over previous
"""Llama GQA attention (B=1, S=2048, E=4096, H=32, KV=8, D=128) on 8 trn2 cores.

Sharding: tensor-parallel over KV groups. Core c owns kv head c and q heads
4c..4c+3: wq/wk/wv output-dim shards, wo input-dim shard. Each core computes a
partial [S, E] output (bf16); host sums the 8 partials and adds bo.

Data plane is bf16 (PE runs bf16 at 1 cycle/row for any moving width; DMA
traffic halves vs f32). PSUM accumulation stays f32; RoPE uses f32 cos/sin.
1/sqrt(D) is folded into wq on the host, so q and k share one cos/sin pair.

Per core, everything transposed [feature, seq]:
  phase 1 (per 512-seq chunk, two passes over resident x tiles so the acc
  PSUM banks double-buffer): q = wq_c.T @ x.T -> 4x [128, S]; k, v -> [128, S].
  RoPE via partition-swapped multiply (host passes sign-adjusted sin).
  v transposed via PE into vTo [k, kt, 129] with a constant 1.0 in col 128.
  phase 2: scoresT tile [k 128, q<=512] = kr.T-matmul; diagonal tiles compute
  only the visible q range and add one shared [128,128] triangle mask; Exp on
  ACT -> P bf16. AV is flipped: stationary = P[:, qs*128:+128], moving =
  vTo[:, kt, 0:129] -> av2 [q 128, 129] accumulates over kt in PSUM, col 128
  = softmax denominator for free. Per-partition reciprocal + scalar-mul
  normalize, PE-transpose back to [D, q] for the o-projection.
  phase 3: out[q, E] += o_tile.T @ wo, interleaved one chunk behind attention
  so the PE keeps busy while ACT catches up on Exp.
PSUM is managed as 8 explicit bank tags in one pool (no pool-boundary stalls):
b0-b2 proj passA / scores+..., b3-b5 proj passB / AV accumulators,
b6-b7 v-transposes / o-proj.
"""

import sys

sys.path.insert(0, "/opt/trn_rl_repo")

import numpy as np
import ml_dtypes

import concourse.bass as bass  # noqa: F401
import concourse.bacc as bacc
import concourse.mybir as mybir
import concourse.tile as tile
from concourse.bass_utils import run_bass_kernel_spmd
from concourse.masks import make_identity

F32 = mybir.dt.float32
BF16 = mybir.dt.bfloat16
ADD = mybir.AluOpType.add
MULT = mybir.AluOpType.mult
EXP = mybir.ActivationFunctionType.Exp
BF = ml_dtypes.bfloat16

B, S, E = 1, 2048, 4096
H, KV, D = 32, 8, 128
NCORES = 8
HPC = H // NCORES          # 4 q heads per core
ET = E // 128              # 32 contraction tiles
SC = S // 512              # 4 seq chunks of 512
KT = S // 128              # 16 k tiles of 128
ECH = E // 512             # 8 output E chunks
NEG = -1e9

SKIP = "S"
NOMASK = "N"

_build_cache = {}


def _build(classes, n_mtiles, use_bias):
    nc = bacc.Bacc(None, target_bir_lowering=False)

    xT = nc.declare_dram_parameter("xT", [E, S], BF16, isOutput=False)
    # wab: [q0|q1|v | q2|q3|k] so pass A's half loads first
    wab = nc.declare_dram_parameter("wab", [E, 6 * D], BF16, isOutput=False)
    wo = nc.declare_dram_parameter("wo", [HPC * D, E], BF16, isOutput=False)
    cos = nc.declare_dram_parameter("cos", [D, S], F32, isOutput=False)
    sinS = nc.declare_dram_parameter("sinS", [D, S], F32, isOutput=False)
    # causal triangle as a rank-128 product: triA.T @ triB = -1e9*(k-q)*[k>q]
    triA = nc.declare_dram_parameter("triA", [D, D], BF16, isOutput=False)
    triB = nc.declare_dram_parameter("triB", [D, D], BF16, isOutput=False)
    if n_mtiles:
        mtiles = nc.declare_dram_parameter(
            "mtiles", [n_mtiles * 128, 512], F32, isOutput=False)
    if use_bias:
        bq = nc.declare_dram_parameter("bq", [HPC * D], F32, isOutput=False)
        bk = nc.declare_dram_parameter("bk", [D], F32, isOutput=False)
        bv = nc.declare_dram_parameter("bv", [D], F32, isOutput=False)
    out = nc.declare_dram_parameter("out", [S, E], BF16, isOutput=True)

    wab_r = wab.rearrange("(t p) n -> p t n", p=128)
    wo_r = wo.rearrange("(t p) n -> p t n", p=128)
    xT_r = xT.rearrange("(t p) s -> p t s", p=128)

    with tile.TileContext(nc) as tc:
        with (
            tc.tile_pool(name="const", bufs=1) as cpool,
            tc.tile_pool(name="qkv", bufs=1) as qkvpool,
            tc.tile_pool(name="wts", bufs=1) as wpool,
            tc.tile_pool(name="xres", bufs=2) as xpool,
            tc.tile_pool(name="cs", bufs=2) as cspool,
            tc.tile_pool(name="tp", bufs=1) as tpool,
            tc.tile_pool(name="ps", bufs=6) as spool,
            tc.tile_pool(name="osb", bufs=2) as opool,
            tc.tile_pool(name="onrm", bufs=4) as onpool,
            tc.tile_pool(name="ob", bufs=4) as obpool,
            tc.tile_pool(name="psum", bufs=1, space="PSUM") as P,
        ):
            ident = cpool.tile([128, 128], BF16)
            make_identity(nc, ident)
            triA_sb = cpool.tile([128, 128], BF16)
            triB_sb = cpool.tile([128, 128], BF16)  # DMA'd after chunk-0 loads
            mt_sb = None
            if n_mtiles:
                mt_sb = cpool.tile([128, n_mtiles, 512], F32)
                nc.sync.dma_start(
                    out=mt_sb,
                    in_=mtiles.rearrange("(t p) n -> p t n", p=128))
            if use_bias:
                bq_sb = cpool.tile([128, HPC], F32)
                nc.sync.dma_start(out=bq_sb, in_=bq.rearrange("(h d) -> d h", d=128))
                bk_sb = cpool.tile([128, 1], F32)
                nc.sync.dma_start(out=bk_sb, in_=bk.rearrange("d -> d 1"))
                bv_sb = cpool.tile([128, 1], F32)
                nc.sync.dma_start(out=bv_sb, in_=bv.rearrange("d -> d 1"))

            # persistent activations
            qr = [qkvpool.tile([128, S], BF16, name=f"qr{h}", tag=f"qr{h}")
                  for h in range(HPC)]
            kr = qkvpool.tile([128, S], BF16, name="kr", tag="kr")
            vTo = qkvpool.tile([128, KT, 129], BF16, tag="vTo")  # [k%128, kt, D|1]
            nc.vector.memset(vTo[:, :, 128:129], 1.0)

            wab_sb = wpool.tile([128, ET, 6 * D], BF16)
            wo_sb = wpool.tile([128, HPC, E], BF16)

            # ---------------- phase 1: projections + RoPE + vT ----------------
            def rope(dst, acc, ct, st_, bias):
                src = acc
                if use_bias:
                    bsrc = tpool.tile([128, 512], F32, name="bsrc", tag="bsrc")
                    nc.vector.tensor_scalar_add(bsrc, acc, bias)
                    src = bsrc
                tmp = tpool.tile([128, 512], F32, name="tmp", tag="tmp")
                nc.vector.tensor_tensor(
                    out=tmp[0:64, :], in0=src[64:128, :], in1=st_[0:64, :], op=MULT)
                nc.vector.tensor_tensor(
                    out=tmp[64:128, :], in0=src[0:64, :], in1=st_[64:128, :], op=MULT)
                tmp2 = tpool.tile([128, 512], F32, name="tmp2", tag="tmp2")
                nc.vector.tensor_tensor(out=tmp2, in0=src, in1=ct, op=MULT)
                nc.vector.tensor_tensor(out=dst, in0=tmp2, in1=tmp, op=ADD)

            BA = ["b0", "b1", "b2"]
            BB = ["b3", "b4", "b5"]
            def emit_vtrans(c, vtmp):
                for j in range(4):
                    tb = "b6" if j % 2 == 0 else "b7"
                    tpsum = P.tile([128, 128], BF16, name="tp", tag=tb)
                    nc.tensor.transpose(
                        tpsum, vtmp[:, j * 128:(j + 1) * 128], ident)
                    nc.vector.tensor_copy(
                        out=vTo[:, c * 4 + j, 0:128], in_=tpsum)

            pend_vtrans = None  # chunk 0's v-transposes run in chunk 1
            for c in range(SC):
                ssl = slice(c * 512, c * 512 + 512)
                xc = xpool.tile([128, ET, 512], BF16, name="xc", tag="xc")
                # DMA in consumption order (HWDGE issue is ~625ns per
                # dma_start — few big DMAs, not many small ones)
                cq = cspool.tile([128, 512], F32, name="cq", tag="cq")
                sq = cspool.tile([128, 512], F32, name="sq", tag="sq")
                if c == 0:
                    # both weight halves per group: chunk 0 runs passes A+B
                    # interleaved so PE outpaces the DMA ramp
                    edges = [0, 1, 4, 8, 12, 16, 20, 24, 28, 32]
                    for gi in range(len(edges) - 1):
                        eg = slice(edges[gi], edges[gi + 1])
                        nc.sync.dma_start(
                            out=wab_sb[:, eg, :], in_=wab_r[:, eg, :])
                        nc.sync.dma_start(out=xc[:, eg, :], in_=xT_r[:, eg, ssl])
                        if gi == 4:
                            nc.sync.dma_start(out=cq, in_=cos[:, ssl])
                            nc.sync.dma_start(out=sq, in_=sinS[:, ssl])
                    nc.sync.dma_start(out=triA_sb, in_=triA[:, :])
                    nc.sync.dma_start(out=triB_sb, in_=triB[:, :])
                else:
                    for g in range(4):
                        eg = slice(g * 8, g * 8 + 8)
                        nc.sync.dma_start(out=xc[:, eg, :], in_=xT_r[:, eg, ssl])
                    nc.sync.dma_start(out=cq, in_=cos[:, ssl])
                    nc.sync.dma_start(out=sq, in_=sinS[:, ssl])
                    if c in (1, 2):
                        for hg in ((0, 1) if c == 1 else (2, 3)):
                            nc.sync.dma_start(
                                out=wo_sb[:, hg:hg + 1, :],
                                in_=wo_r[:, hg:hg + 1, :])

                accs = [P.tile([128, 512], F32, name=f"pa{i}", tag=BA[i])
                        for i in range(3)]
                accs2 = [P.tile([128, 512], F32, name=f"pb{i}", tag=BB[i])
                         for i in range(3)]
                def make_vtmp(acc):
                    vt = tpool.tile([128, 512], BF16, name="vtmp", tag="vtmp",
                                    bufs=2)
                    if use_bias:
                        nc.vector.tensor_scalar_add(vt, acc, bv_sb[:, 0:1])
                    else:
                        nc.scalar.copy(out=vt, in_=acc)
                    return vt

                if c == 0:
                    # single fused pass, e-major: chunk 0 is DMA-paced so the
                    # matmuls must chase the per-e loads
                    for e in range(ET):
                        st, sp = (e == 0), (e == ET - 1)
                        for ai, col in enumerate((0, 128, 256)):
                            nc.tensor.matmul(
                                accs[ai], wab_sb[:, e, col:col + 128],
                                xc[:, e, :], start=st, stop=sp)
                        for ai, col in enumerate((384, 512, 640)):
                            nc.tensor.matmul(
                                accs2[ai], wab_sb[:, e, col:col + 128],
                                xc[:, e, :], start=st, stop=sp)
                    rope(qr[0][:, ssl], accs[0], cq, sq,
                         bq_sb[:, 0:1] if use_bias else None)
                    rope(qr[1][:, ssl], accs[1], cq, sq,
                         bq_sb[:, 1:2] if use_bias else None)
                    vtmp = make_vtmp(accs[2])
                    pend_vtrans = (0, vtmp)
                    rope(kr[:, ssl], accs2[2], cq, sq,
                         bk_sb[:, 0:1] if use_bias else None)
                    rope(qr[2][:, ssl], accs2[0], cq, sq,
                         bq_sb[:, 2:3] if use_bias else None)
                    rope(qr[3][:, ssl], accs2[1], cq, sq,
                         bq_sb[:, 3:4] if use_bias else None)
                else:
                    # acc-major: each accumulator finishes early so its RoPE /
                    # copy overlaps the next accumulation instead of tailing
                    def acc_loop(acc, col):
                        for e in range(ET):
                            nc.tensor.matmul(
                                acc, wab_sb[:, e, col:col + 128], xc[:, e, :],
                                start=(e == 0), stop=(e == ET - 1))
                    acc_loop(accs[2], 256)                     # v
                    vtmp = make_vtmp(accs[2])
                    if pend_vtrans is not None:
                        emit_vtrans(*pend_vtrans)
                        pend_vtrans = None
                    acc_loop(accs[0], 0)                       # q0
                    rope(qr[0][:, ssl], accs[0], cq, sq,
                         bq_sb[:, 0:1] if use_bias else None)
                    acc_loop(accs[1], 128)                     # q1
                    rope(qr[1][:, ssl], accs[1], cq, sq,
                         bq_sb[:, 1:2] if use_bias else None)
                    emit_vtrans(c, vtmp)
                    if c == SC - 1:
                        acc_loop(accs2[2], 640)                # k first
                        rope(kr[:, ssl], accs2[2], cq, sq,
                             bk_sb[:, 0:1] if use_bias else None)
                        acc_loop(accs2[0], 384)                # q2
                        acc_loop(accs2[1], 512)                # q3
                        # defer the last two RoPEs: they would block q-chunk-0
                        # attention's DVE work (in-order engine)
                        pend_ropes = [
                            (qr[2][:, ssl], accs2[0],
                             bq_sb[:, 2:3] if use_bias else None),
                            (qr[3][:, ssl], accs2[1],
                             bq_sb[:, 3:4] if use_bias else None),
                        ]
                        pend_rope_cs = (cq, sq)
                    else:
                        acc_loop(accs2[0], 384)                # q2
                        rope(qr[2][:, ssl], accs2[0], cq, sq,
                             bq_sb[:, 2:3] if use_bias else None)
                        acc_loop(accs2[1], 512)                # q3
                        rope(qr[3][:, ssl], accs2[1], cq, sq,
                             bq_sb[:, 3:4] if use_bias else None)
                        acc_loop(accs2[2], 640)                # k
                        rope(kr[:, ssl], accs2[2], cq, sq,
                             bk_sb[:, 0:1] if use_bias else None)

            # ------------- phases 2+3: attention (+interleaved o-proj) -------
            o_bufs = [None, None]  # [qc%2] -> list of 4 o_sb tiles

            def oproj_pair(qcp, qs, ec0, slot=("b6", "b7"),
                           acts=(False, False), split_dma=False):
                """Two consecutive ec units sharing one output DMA."""
                osrc = o_bufs[qcp % 2]
                q0 = qcp * 512 + qs * 128
                ob = obpool.tile([128, 1024], BF16, name="ob", tag="ob")
                for k in range(2):
                    ec = ec0 + k
                    op = P.tile([128, 512], F32, name="op", tag=slot[k])
                    for hh in range(HPC):
                        nc.tensor.matmul(
                            op,
                            osrc[hh][:, qs * 128:(qs + 1) * 128],
                            wo_sb[:, hh, ec * 512:(ec + 1) * 512],
                            start=(hh == 0), stop=(hh == HPC - 1),
                            skip_group_check=True)
                    if acts[k]:
                        nc.scalar.copy(out=ob[:, k * 512:(k + 1) * 512], in_=op)
                    else:
                        nc.vector.tensor_copy(
                            out=ob[:, k * 512:(k + 1) * 512], in_=op)
                    if split_dma:
                        nc.sync.dma_start(
                            out=out[q0:q0 + 128, ec * 512:(ec + 1) * 512],
                            in_=ob[:, k * 512:(k + 1) * 512])
                if not split_dma:
                    nc.sync.dma_start(
                        out=out[q0:q0 + 128, ec0 * 512:(ec0 + 2) * 512], in_=ob)

            for qc in range(SC):
                # qc0 runs before the deferred chunk-3 RoPE tail frees b3/b4;
                # b5 is free (its k RoPE ran inline during chunk 3)
                AVB = (["b2", "b6", "b7", "b5"] if qc == 0
                       else ["b2", "b3", "b4", "b5"])
                cls = classes[qc]
                vis = [kt for kt in range(KT) if cls[kt] != SKIP]
                o_cur = [opool.tile([128, 512], BF16, name=f"o{h}", tag=f"o{h}")
                         for h in range(HPC)]
                o_bufs[qc % 2] = o_cur
                for h in range(HPC):
                    pend_oproj = list(range(ECH)) if qc > 0 else []
                    ob_half = [None]  # open ob tile for the current pair

                    def emit_op_unit(ec):
                        """One o-proj ec unit; pairs share an ob tile+DMA."""
                        osrc = o_bufs[(qc - 1) % 2]
                        q0 = (qc - 1) * 512 + h * 128
                        if ec % 2 == 0:
                            ob_half[0] = obpool.tile(
                                [128, 1024], BF16, name="ob", tag="ob")
                        ob = ob_half[0]
                        op = P.tile([128, 512], F32, name="op",
                                    tag="b6" if ec % 2 == 0 else "b7")
                        for hh in range(HPC):
                            nc.tensor.matmul(
                                op,
                                osrc[hh][:, h * 128:(h + 1) * 128],
                                wo_sb[:, hh, ec * 512:(ec + 1) * 512],
                                start=(hh == 0), stop=(hh == HPC - 1),
                                skip_group_check=True)
                        k = ec % 2
                        if k == 0:
                            nc.scalar.copy(
                                out=ob[:, k * 512:(k + 1) * 512], in_=op)
                        else:
                            nc.vector.tensor_copy(
                                out=ob[:, k * 512:(k + 1) * 512], in_=op)
                        if k == 1:
                            nc.sync.dma_start(
                                out=out[q0:q0 + 128,
                                        (ec - 1) * 512:(ec + 1) * 512],
                                in_=ob)
                    # per-qs AV accumulation state: kt lists
                    avkts = [[kt for kt in vis
                              if not (isinstance(cls[kt], tuple)
                                      and cls[kt][0] == "T"
                                      and cls[kt][1] > qs)]
                             for qs in range(4)]
                    avseen = [0, 0, 0, 0]
                    av2 = [P.tile([128, 129], F32, name=f"av{qs}", tag=AVB[qs])
                           for qs in range(4)]
                    def emit_qs_tail(qs):
                        rl = onpool.tile([128, 1], F32, name="rl", tag="rl")
                        nc.vector.reciprocal(rl, av2[qs][:, 128:129])
                        otn = onpool.tile([128, 128], BF16, name="otn",
                                          tag="otn")
                        nc.vector.tensor_scalar_mul(otn, av2[qs][:, 0:128], rl)
                        tp2 = P.tile([128, 128], BF16, name="tp2", tag=AVB[qs])
                        nc.tensor.transpose(tp2, otn, ident)
                        if qc >= 2:  # ACT is exp-saturated in late chunks
                            nc.vector.tensor_copy(
                                out=o_cur[h][:, qs * 128:(qs + 1) * 128],
                                in_=tp2)
                        else:
                            nc.scalar.copy(
                                out=o_cur[h][:, qs * 128:(qs + 1) * 128],
                                in_=tp2)

                    def emit_av(kt, p):
                        for qs in range(4):
                            if kt not in avkts[qs]:
                                continue
                            first = avseen[qs] == 0
                            avseen[qs] += 1
                            last = avseen[qs] == len(avkts[qs])
                            nc.tensor.matmul(
                                av2[qs],
                                p[:, qs * 128:(qs + 1) * 128],
                                vTo[:, kt, :],
                                start=first, stop=last,
                                skip_group_check=True)
                            if last:
                                tails.append(qs)

                    pops = {}
                    for k in range(ECH):
                        pops.setdefault((k * len(vis)) // ECH, 0)
                        pops[(k * len(vis)) // ECH] += 1
                    prev = None  # (kt, p) — AV runs one tile behind exp
                    tails = []  # qs normalize/transpose, deferred one unit
                    for i, kt in enumerate(vis):
                        if tails:
                            emit_qs_tail(tails.pop(0))
                        cl = cls[kt]
                        q0 = cl[1] * 128 if (isinstance(cl, tuple)
                                             and cl[0] == "T") else 0
                        stp = P.tile([128, 512], F32, name="st",
                                     tag="b0" if i % 2 == 0 else "b1")
                        nc.tensor.matmul(
                            stp[:, q0:512],
                            kr[:, kt * 128:(kt + 1) * 128],
                            qr[h][:, qc * 512 + q0: qc * 512 + 512],
                            start=True, stop=True, skip_group_check=True)
                        if isinstance(cl, tuple) and cl[0] == "T":
                            nc.tensor.matmul(
                                stp[:, q0:q0 + 128], triA_sb, triB_sb,
                                start=False, stop=True, skip_group_check=True)
                        elif isinstance(cl, tuple) and cl[0] == "M":
                            nc.vector.tensor_tensor(
                                out=stp, in0=stp, in1=mt_sb[:, cl[1], :],
                                op=ADD)
                        p = spool.tile([128, 512], BF16, name="p", tag="p")
                        nc.scalar.activation(
                            out=p[:, q0:512], in_=stp[:, q0:512], func=EXP)
                        if prev is not None:
                            emit_av(*prev)
                        prev = (kt, p)
                        for _ in range(pops.get(i, 0)):
                            if pend_oproj:
                                emit_op_unit(pend_oproj.pop(0))
                    emit_av(*prev)
                    while tails or pend_oproj:
                        if pend_oproj:
                            emit_op_unit(pend_oproj.pop(0))
                        if tails:
                            emit_qs_tail(tails.pop(0))
                if qc == 0:
                    cqd, sqd = pend_rope_cs
                    for dst, acc, bias in pend_ropes:
                        rope(dst, acc, cqd, sqd, bias)
            # final o-proj for the last chunk: six banks, copies alternate
            # DVE/ACT (nothing else runs here)
            FB = ["b0", "b1", "b2", "b3", "b4", "b5"]
            for qs in range(4):
                for pi, ec0 in enumerate(range(0, ECH, 2)):
                    u = qs * 4 + pi
                    oproj_pair(SC - 1, qs, ec0,
                               slot=(FB[(2 * u) % 6], FB[(2 * u + 1) % 6]),
                               acts=(False, True), split_dma=(u >= 14))

    nc.finalize()
    return nc


def _host_prep(x, mask, position_ids, wq, bq, wk, bk, wv, bv, wo, bo):
    scale = 1.0 / np.sqrt(np.float32(D))
    xT = np.ascontiguousarray(x.reshape(S, E).T).astype(BF)
    wq_s = (wq * scale).astype(np.float32)
    wk_b = wk.astype(np.float32)
    wv_b = wv.astype(np.float32)
    wo_b = wo.astype(BF)

    pos = position_ids.reshape(S).astype(np.float32)
    inv_freq = 1.0 / (10000.0 ** (np.arange(0, D, 2, dtype=np.float32) / D))
    freqs = np.outer(pos, inv_freq)                     # [S, D/2]
    emb = np.concatenate([freqs, freqs], axis=1)        # [S, D]
    cosT = np.ascontiguousarray(np.cos(emb).astype(np.float32).T)
    sin = np.sin(emb).astype(np.float32)
    sin[:, : D // 2] *= -1.0                            # sign for partition swap
    sinT = np.ascontiguousarray(sin.T)

    maskT = np.ascontiguousarray(mask.reshape(S, S).T)
    # canonical 128x128 triangle: T[k, q] = 0 if q >= k else NEG
    ktri = np.arange(128)[:, None]
    qtri = np.arange(128)[None, :]
    tri = np.where(qtri >= ktri, 0.0, NEG).astype(np.float32)
    # rank-128 factors: (triA.T @ triB)[k, q] = -s^2 (k - q) for k > q, 0 else
    sfac = np.float32(np.sqrt(1e9))
    mtri = np.arange(128)
    triA = np.where(mtri[:, None] < mtri[None, :], -sfac, 0.0).astype(BF)
    triB = np.where(mtri[:, None] >= mtri[None, :], sfac, 0.0).astype(BF)

    classes = []
    muniq = []      # unique general mask tiles
    mkeys = {}

    def mref(t):
        key = t.tobytes()
        if key not in mkeys:
            mkeys[key] = len(muniq)
            muniq.append(t)
        return ("M", mkeys[key])

    for qc in range(SC):
        row = []
        for kt in range(KT):
            t = maskT[kt * 128:(kt + 1) * 128, qc * 512:qc * 512 + 512]
            if np.all(t <= -1e8):
                row.append(SKIP)
                continue
            if np.all(t == 0.0):
                row.append(NOMASK)
                continue
            j = kt - 4 * qc
            if 0 <= j <= 3:
                q0 = j * 128
                ok = (q0 == 0 or np.all(t[:, :q0] <= -1e8))
                ok = ok and np.array_equal(t[:, q0:q0 + 128], tri)
                ok = ok and (q0 + 128 == 512 or np.all(t[:, q0 + 128:] == 0.0))
                if ok:
                    row.append(("T", j))
                    continue
            row.append(mref(t))
        if all(c == SKIP for c in row):       # fully-masked rows: keep finite
            row = [mref(maskT[kt * 128:(kt + 1) * 128,
                              qc * 512:qc * 512 + 512]) for kt in range(KT)]
        classes.append(tuple(row))
    classes = tuple(classes)

    mtiles = np.concatenate(muniq, axis=0) if muniq else None
    use_bias = bool(np.any(bq) or np.any(bk) or np.any(bv))
    return (xT, wq_s, wk_b, wv_b, wo_b, cosT, sinT, triA, triB, mtiles,
            classes, use_bias)


def kernel(x, mask, position_ids, wq, bq, wk, bk, wv, bv, wo, bo):
    (xT, wq_s, wk_b, wv_b, wo_b, cosT, sinT, triA, triB, mtiles,
     classes, use_bias) = _host_prep(
        x, mask, position_ids, wq, bq, wk, bk, wv, bv, wo, bo)

    n_mtiles = 0 if mtiles is None else mtiles.shape[0] // 128
    key = (classes, n_mtiles, use_bias)
    if key not in _build_cache:
        _build_cache[key] = _build(classes, n_mtiles, use_bias)
    nc = _build_cache[key]

    in_maps = []
    for c in range(NCORES):
        qsl = slice(c * HPC * D, (c + 1) * HPC * D)
        ksl = slice(c * D, (c + 1) * D)
        wqc = wq_s[:, qsl]
        # [q0|q1|v | q2|q3|k]: pass-A half first
        wab = np.ascontiguousarray(np.concatenate(
            [wqc[:, 0:256], wv_b[:, ksl], wqc[:, 256:512], wk_b[:, ksl]],
            axis=1)).astype(BF)
        m = {
            "xT": xT,
            "wab": wab,
            "wo": np.ascontiguousarray(wo_b[qsl, :]),
            "cos": cosT, "sinS": sinT, "triA": triA, "triB": triB,
        }
        if mtiles is not None:
            m["mtiles"] = mtiles
        if use_bias:
            m["bq"] = np.ascontiguousarray(bq[qsl]).astype(np.float32)
            m["bk"] = np.ascontiguousarray(bk[ksl]).astype(np.float32)
            m["bv"] = np.ascontiguousarray(bv[ksl]).astype(np.float32)
        in_maps.append(m)

    res = run_bass_kernel_spmd(nc, in_maps, list(range(NCORES)))
    kernel._last_results = res

    acc = res.results[0]["out"].astype(np.float32)
    for c in range(1, NCORES):
        acc = acc + res.results[c]["out"].astype(np.float32)
    acc = acc + bo[None, :]
    return acc.reshape(B, S, E).astype(np.float32)
